# revision 1
# baseline (speedup 1.0000x reference)
"""Trainium2 Bass kernel for nn_MultiHeadAttention_42640435315371.

Data-parallel over 8 NeuronCores: each core handles 2048 of the 16384
(n*t) tokens; the four d_model x d_model weights are replicated (shipped
bf16, pre-transposed/permuted on host).

Math notes (matching reference.py exactly):
  - energy_t = Qh_t^T Kh_t / 32 per token (token-local "attention");
    the 1/32 scale and the mask are folded into K as K * mask/32, so a
    masked token yields an all-zero energy matrix -> softmax = uniform
    1/64, identical to softmax of a constant -1e20 row.
  - energies are tiny (|E| < ~1), so exp() needs no max-subtraction.
  - concat order is (d_head, head); Wo's columns are permuted on host so
    the device can emit rows k = h*64 + i.
"""

import numpy as np

import concourse.bass as bass
import concourse.mybir as mybir
from concourse import bacc
from concourse.tile import TileContext
from concourse.bass_utils import run_bass_kernel_spmd

F32 = mybir.dt.float32
BF16 = mybir.dt.bfloat16

N_CORES = 8
N, T, D, H, DH = 4, 4096, 1024, 16, 64
TOK = (N * T) // N_CORES  # 2048 tokens per core
MT = 512                  # megatile tokens
import os
NMT = int(os.environ.get('K_NMT', TOK // MT))
STAGE = int(os.environ.get('K_STAGE', 99))
SUB = int(os.environ.get('K_SUB', 99))


def build_nc():
    nc = bacc.Bacc("TRN2", target_bir_lowering=False, debug=False,
                   num_devices=N_CORES)
    xq = nc.declare_dram_parameter("xq", [D, TOK], F32, isOutput=False)
    xk = nc.declare_dram_parameter("xk", [D, TOK], F32, isOutput=False)
    xv = nc.declare_dram_parameter("xv", [D, TOK], F32, isOutput=False)
    wq = nc.declare_dram_parameter("wq", [D, D], BF16, isOutput=False)
    wk = nc.declare_dram_parameter("wk", [D, D], BF16, isOutput=False)
    wv = nc.declare_dram_parameter("wv", [D, D], BF16, isOutput=False)
    wo = nc.declare_dram_parameter("wo", [D, D], BF16, isOutput=False)
    m32 = nc.declare_dram_parameter("m32", [128, TOK // 128], F32, isOutput=False)
    ident = nc.declare_dram_parameter("ident", [128, 128], BF16, isOutput=False)
    out = nc.declare_dram_parameter("out", [D, TOK], F32, isOutput=True)

    from contextlib import ExitStack
    with TileContext(nc) as tc, ExitStack() as ctx:
        const = ctx.enter_context(tc.tile_pool(name="const", bufs=1))
        p_xb = ctx.enter_context(tc.tile_pool(name="xb", bufs=8))
        p_maj = ctx.enter_context(tc.tile_pool(name="maj", bufs=2))
        p_cc = ctx.enter_context(tc.tile_pool(name="cc", bufs=8))
        p_exp = ctx.enter_context(tc.tile_pool(name="expp", bufs=2))
        p_hd = ctx.enter_context(tc.tile_pool(name="hd", bufs=2))
        p_rcp = ctx.enter_context(tc.tile_pool(name="rcp", bufs=2))
        p_outT = ctx.enter_context(tc.tile_pool(name="outT", bufs=2))
        ps_proj = ctx.enter_context(tc.tile_pool(name="psp", bufs=2, space="PSUM"))
        ps_E = ctx.enter_context(tc.tile_pool(name="psE", bufs=3, space="PSUM"))
        ps_2 = ctx.enter_context(tc.tile_pool(name="ps2", bufs=2, space="PSUM"))
        p_stage = ctx.enter_context(tc.tile_pool(name="stage", bufs=2,
                                                 space="DRAM"))

        # ---- static tiles ----
        def load_w(name, dram):
            tiles = []
            for i in range(8):
                t = const.tile([128, D], BF16, tag=f"{name}{i}")
                nc.sync.dma_start(out=t[:], in_=dram[i * 128:(i + 1) * 128, :])
                tiles.append(t)
            return tiles

        wq_sb, wk_sb, wv_sb, wo_sb = (load_w(n, d) for n, d in
                                      (("wq", wq), ("wk", wk), ("wv", wv), ("wo", wo)))
        m32_sb = const.tile([128, TOK // 128], F32, tag="m32")
        nc.sync.dma_start(out=m32_sb[:], in_=m32[:])
        id_sb = const.tile([128, 128], BF16, tag="ident")
        nc.sync.dma_start(out=id_sb[:], in_=ident[:])
        ones_bd = const.tile([128, 2], BF16, tag="onesbd")
        nc.vector.memset(ones_bd[:], 0.0)
        nc.vector.memset(ones_bd[0:64, 0:1], 1.0)
        nc.vector.memset(ones_bd[64:128, 1:2], 1.0)
        # Packed per-tc4 shuffle tiles (2x ping-pong, zeros/ones static).
        # stqT [64=(g2,b,h), 32gf*64i]; bdkT [64=(g2,b,h), 32gf*128(y,j)]
        # block-diagonal in (b,y); bdvT [128=(b,j), 64g*34(b',h | ones)].
        stqT_pp, bdkT_pp, bdvT_pp = [], [], []
        for i in range(2):
            t = const.tile([32, 64 * 64], BF16, tag=f"stqT{i}")
            stqT_pp.append(t)
            t = const.tile([32, 64 * 128], BF16, tag=f"bdkT{i}")
            nc.vector.memset(t[:], 0.0)
            bdkT_pp.append(t)
            t = const.tile([128, 64 * 34], BF16, tag=f"bdvT{i}")
            nc.vector.memset(t[:], 0.0)
            for b in range(2):
                # ones column at (b',h)-col 32+b for row-half b
                nc.vector.memset(
                    t[b * 64:(b + 1) * 64, :].rearrange(
                        "j (g c) -> j g c", c=34)[:, :, 32 + b:33 + b], 1.0)
            bdvT_pp.append(t)

        Copy = mybir.ActivationFunctionType.Copy
        Exp = mybir.ActivationFunctionType.Exp
        Mult = mybir.AluOpType.mult

        for mt in range(NMT):
            t0 = mt * MT
            # ---- load x megatile, cast to bf16 ----
            def load_x(dram, name):
                sbs = []
                for kc in range(8):
                    tb = p_xb.tile([128, MT], BF16, tag=f"x{name}")
                    # gpsimd DMA casts f32 -> bf16 in flight
                    nc.gpsimd.dma_start(out=tb[:],
                                        in_=dram[kc * 128:(kc + 1) * 128,
                                                 t0:t0 + MT])
                    sbs.append(tb)
                return sbs

            xq_sb = load_x(xq, "q")
            xk_sb = load_x(xk, "k")
            xv_sb = load_x(xv, "v")

            # ---- projections (T-major: out[t_chunk, o]) ----
            qmaj, kmaj, vmaj = [], [], []
            for tc4 in range(4):
                qm = p_maj.tile([128, D], BF16, tag="qmaj")
                km = p_maj.tile([128, D], BF16, tag="kmaj")
                vm = p_maj.tile([128, D], BF16, tag="vmaj")
                for dst, xsb, wsb, is_k in ((qm, xq_sb, wq_sb, False),
                                            (km, xk_sb, wk_sb, True),
                                            (vm, xv_sb, wv_sb, False)):
                    pss = [ps_proj.tile([128, 512], F32, tag="psp",
                                        name=f"psp{mt}_{tc4}_{id(dst)}_{i}")
                           for i in range(2)]
                    for kc in range(8):
                        for oc2 in range(2):
                            nc.tensor.matmul(
                                out=pss[oc2][:],
                                lhsT=xsb[kc][:, tc4 * 128:(tc4 + 1) * 128],
                                rhs=wsb[kc][:, oc2 * 512:(oc2 + 1) * 512],
                                start=(kc == 0), stop=(kc == 7))
                    for oc2 in range(2):
                        dslice = dst[:, oc2 * 512:(oc2 + 1) * 512]
                        if is_k:
                            mcol = mt * 4 + tc4
                            nc.vector.tensor_scalar(
                                out=dslice, in0=pss[oc2][:],
                                scalar1=m32_sb[:, mcol:mcol + 1], scalar2=None,
                                op0=Mult)
                        else:
                            nc.scalar.activation(out=dslice, in_=pss[oc2][:],
                                                 func=Copy)
                qmaj.append(qm)
                kmaj.append(km)
                vmaj.append(vm)

            if STAGE <= 1:
                for oc in range(8):
                    nc.gpsimd.dma_start(out=out[oc * 128:(oc + 1) * 128,
                                                t0:t0 + MT],
                                        in_=qmaj[oc % 4][:, 0:512])
                continue
            # ---- V fold via DRAM staging: bdvT built per tc4 below ----

            # ---- attention ----
            concatT = [p_cc.tile([128, MT], F32, tag="cc", name=f"cc{mt}_{i}")
                       for i in range(8)]
            rcp64 = p_rcp.tile([64, MT], F32, tag="rcp64")
            sh = p_stage.tile([16, 64, MT], F32, tag="sh")  # [h][i][t]
            for tc4 in range(4):
                pp = tc4 % 2
                stqT, bdkT, bdvT = stqT_pp[pp], bdkT_pp[pp], bdvT_pp[pp]
                # -- stage Q/K through DRAM to build packed shuffle tiles --
                sq = p_stage.tile([2, 2, 16, 32, 64], BF16, tag="sq")
                sk = p_stage.tile([2, 2, 16, 32, 64], BF16, tag="sk")
                qv4 = qmaj[tc4][:].rearrange("(gf g2 b) (h i) -> g2 b gf h i",
                                             gf=32, g2=2, i=64)
                kv4 = kmaj[tc4][:].rearrange("(gf g2 b) (h j) -> g2 b gf h j",
                                             gf=32, g2=2, j=64)
                for g2 in range(2):
                    for b in range(2):
                        nc.sync.dma_start(
                            out=sq[g2, b].rearrange("h gf i -> gf h i"),
                            in_=qv4[g2, b])
                        nc.sync.dma_start(
                            out=sk[g2, b].rearrange("h gf j -> gf h j"),
                            in_=kv4[g2, b])
                for g2 in range(2):
                    nc.sync.dma_start(
                        out=stqT[:].rearrange("p (gf g2 i) -> p g2 gf i",
                                              g2=2, i=64)[:, g2],
                        in_=sq[g2])
                    for b in range(2):
                        nc.sync.dma_start(
                            out=bdkT[b * 16:(b + 1) * 16, :].rearrange(
                                "h (gf g2 y j) -> h g2 gf y j",
                                g2=2, y=2, j=64)[:, g2, :, b, :],
                            in_=sk[g2, b])
                # -- bdvT via DRAM staging: S_v[b][j][g][h] --
                sv = p_stage.tile([2, 64, 64, 16], BF16, tag="sv")
                # vmaj is h-innermost (wv host-permuted): cols = j*16+h
                vv = vmaj[tc4][:].rearrange("(g b) (j h) -> b g j h", b=2, h=16)
                for b in range(2):
                    nc.scalar.dma_start(
                        out=sv[b].rearrange("j g h -> g j h"), in_=vv[b])
                    nc.scalar.dma_start(
                        out=bdvT[b * 64:(b + 1) * 64, :].rearrange(
                            "j (g c) -> j g c", c=34)[:, :, b * 16:(b + 1) * 16],
                        in_=sv[b])
                if STAGE <= 2:
                    continue
                for batch in range(8):  # 16 tokens
                    bt = tc4 * 8 + batch
                    psE = ps_E.tile([128, 512], F32, tag="psE")
                    for g8 in range(8):
                        g = batch * 8 + g8      # group in tc4 (2 tokens)
                        nc.tensor.matmul(
                            out=psE[:, g8 * 64:(g8 + 1) * 64],
                            lhsT=bdkT[:, g * 128:(g + 1) * 128],
                            rhs=stqT[:, g * 64:(g + 1) * 64],
                            start=True, stop=True)
                    expE = p_exp.tile([128, 512], BF16, tag="expE")
                    nc.scalar.activation(out=expE[:], in_=psE[:], func=Exp)
                    if SUB <= 1:
                        if batch == 0:
                            nc.gpsimd.dma_start(
                                out=sh[:, :, bt * 16:(bt + 1) * 16].rearrange(
                                    "h i t -> i h t"),
                                in_=expE[0:64, 0:256].rearrange(
                                    "i (h t) -> i h t", h=16))
                        continue
                    ps2 = ps_2.tile([64, 272], F32, tag="ps2")
                    for g8 in range(8):
                        g = batch * 8 + g8
                        nc.tensor.matmul(
                            out=ps2[:, g8 * 34:(g8 + 1) * 34],
                            lhsT=expE[:, g8 * 64:(g8 + 1) * 64],
                            rhs=bdvT[:, g * 34:(g + 1) * 34],
                            start=True, stop=True)
                    if SUB <= 2:
                        if batch == 0:
                            nc.scalar.activation(out=rcp64[:, bt*16:(bt+1)*16],
                                                 in_=ps2[:, 0:16], func=Copy)
                        continue
                    ps2v = ps2[:].rearrange("p (g c) -> p g c", c=34)
                    nc.vector.reciprocal(
                        rcp64[:, bt * 16:(bt + 1) * 16].rearrange(
                            "p (g b) -> p g b", b=2),
                        ps2v[:, :, 32:34])
                    # hd cols (h,g,b) <- ps2 cols (g,(b,h)); matching 4D walks
                    hd = p_hd.tile([64, 256], F32, tag="hd")
                    nc.scalar.activation(
                        out=hd[:].rearrange("p (h g b) -> p g b h",
                                            h=16, g=8, b=2),
                        in_=ps2v[:, :, 0:32].rearrange("p g (b h) -> p g b h",
                                                       h=16),
                        func=Copy)
                    # stage head rows: sh[h][i][t-slice]
                    nc.scalar.dma_start(
                        out=sh[:, :, bt * 16:(bt + 1) * 16].rearrange(
                            "h i t -> i h t"),
                        in_=hd[:].rearrange("i (h t) -> i h t", h=16))
            if STAGE <= 2:
                for oc in range(8):
                    nc.gpsimd.dma_start(out=out[oc * 128:(oc + 1) * 128,
                                                t0:t0 + MT],
                                        in_=bdkT_pp[oc % 2][:, 0:1024])
                continue
            for kc in range(8):
                nc.scalar.dma_start(out=concatT[kc][:],
                                    in_=sh[2 * kc:2 * kc + 2])
            if STAGE <= 3:
                for oc in range(8):
                    nc.sync.dma_start(out=out[oc * 128:(oc + 1) * 128,
                                              t0:t0 + MT],
                                      in_=concatT[oc][:])
                continue

            # ---- normalize + output projection ----
            rcp128 = p_rcp.tile([128, MT], F32, tag="rcp128")
            nc.vector.tensor_copy(rcp128[0:64, :], rcp64[:])
            nc.sync.dma_start(out=rcp128[64:128, :], in_=rcp64[:])
            ccb = []
            for kc in range(8):
                cb = p_cc.tile([128, MT], BF16, tag="ccb")
                nc.vector.tensor_tensor(out=cb[:], in0=concatT[kc][:],
                                        in1=rcp128[:], op=Mult)
                ccb.append(cb)
            for oc in range(8):
                ps = ps_proj.tile([128, 512], F32, tag="psp")
                for kc in range(8):
                    nc.tensor.matmul(out=ps[:],
                                     lhsT=wo_sb[kc][:, oc * 128:(oc + 1) * 128],
                                     rhs=ccb[kc][:],
                                     start=(kc == 0), stop=(kc == 7))
                ot = p_outT.tile([128, MT], F32, tag="outT")
                nc.scalar.activation(out=ot[:], in_=ps[:], func=Copy)
                nc.sync.dma_start(out=out[oc * 128:(oc + 1) * 128, t0:t0 + MT],
                                  in_=ot[:])
    nc.compile()
    return nc


_NC_CACHE = None


def _get_nc():
    global _NC_CACHE
    if _NC_CACHE is None:
        _NC_CACHE = build_nc()
    return _NC_CACHE


def _host_prep(queries, keys, values, mask, Wq, Wk, Wv, Wo):
    """Build the 8 per-core input maps."""
    fq = np.ascontiguousarray(queries.reshape(N * T, D).T)  # [D, 16384]
    fk = np.ascontiguousarray(keys.reshape(N * T, D).T)
    fv = np.ascontiguousarray(values.reshape(N * T, D).T)
    fm = mask.reshape(N * T).astype(np.float32) / 32.0

    import ml_dtypes
    bf = lambda a: np.ascontiguousarray(a).astype(ml_dtypes.bfloat16)
    wq_h = bf(Wq.T)
    wk_h = bf(Wk.T)
    ov = np.arange(D)
    perm_v = (ov % 16) * 64 + (ov // 16)  # device col j*16+h <- orig h*64+j
    wv_h = bf(Wv.T[:, perm_v])
    kpp = np.arange(D)
    perm = (kpp % 64) * 16 + (kpp // 64)  # k''=h*64+i -> source row i*16+h
    wo_h = bf(Wo.T[perm])
    ident = np.eye(128, dtype=np.float32).astype(ml_dtypes.bfloat16)

    in_maps = []
    for c in range(N_CORES):
        s = slice(c * TOK, (c + 1) * TOK)
        in_maps.append({
            "xq": np.ascontiguousarray(fq[:, s]),
            "xk": np.ascontiguousarray(fk[:, s]),
            "xv": np.ascontiguousarray(fv[:, s]),
            "wq": wq_h, "wk": wk_h, "wv": wv_h, "wo": wo_h,
            "m32": np.ascontiguousarray(fm[s].reshape(TOK // 128, 128).T),
            "ident": ident,
        })
    return in_maps


def kernel(queries, keys, values, mask, Wq, Wk, Wv, Wo, _trace=False, _tmpdir=None):
    queries = np.asarray(queries, dtype=np.float32)
    keys = np.asarray(keys, dtype=np.float32)
    values = np.asarray(values, dtype=np.float32)
    mask = np.asarray(mask)
    in_maps = _host_prep(queries, keys, values, mask,
                         np.asarray(Wq, np.float32), np.asarray(Wk, np.float32),
                         np.asarray(Wv, np.float32), np.asarray(Wo, np.float32))
    nc = _get_nc()
    res = run_bass_kernel_spmd(nc, in_maps, core_ids=list(range(N_CORES)),
                               trace=_trace, tmpdir=_tmpdir)
    outs = []
    for c in range(N_CORES):
        outs.append(np.asarray(res.results[c]["out"]).T)  # [TOK, D]
    full = np.concatenate(outs, axis=0).reshape(N, T, D)
    kernel.last_exec_time_ns = res.exec_time_ns
    return full



# revision 10
# speedup vs baseline: 1.2980x; 1.2980x over previous
"""Trainium2 Bass kernel for nn_MultiHeadAttention_42640435315371.

Data-parallel over 8 NeuronCores: each core handles 2048 of the 16384
(n*t) tokens; the four d_model x d_model weights are replicated (shipped
bf16, pre-transposed/permuted on host).

Math notes (matching reference.py exactly):
  - energy_t = Qh_t^T Kh_t / 32 per token (token-local "attention");
    the 1/32 scale and the mask are folded into K as K * mask/32, so a
    masked token yields an all-zero energy matrix -> softmax = uniform
    1/64, identical to softmax of a constant -1e20 row.
  - energies are tiny (|E| < ~1), so exp() needs no max-subtraction.
  - concat order is (d_head, head); Wo's columns are permuted on host so
    the device can emit rows k = h*64 + i.

v2 scheduling: DMA staging merged into few large transfers; DRAM staging
buffers written in source order (sequential writes, strided reads);
engine split: ACT=exp only, DVE=copies/normalize, sync-ring=Q/K/V staging,
SWDGE(gpsimd)=x loads + head staging; deeper pools for cross-megatile
overlap.
"""

import numpy as np

import concourse.bass as bass
import concourse.mybir as mybir
from concourse import bacc
from concourse.tile import TileContext
from concourse.bass_utils import run_bass_kernel_spmd

F32 = mybir.dt.float32
BF16 = mybir.dt.bfloat16

N_CORES = 8
N, T, D, H, DH = 4, 4096, 1024, 16, 64
TOK = (N * T) // N_CORES  # 2048 tokens per core
MT = 512                  # megatile tokens
import os
NMT = int(os.environ.get('K_NMT', TOK // MT))
STAGE = int(os.environ.get('K_STAGE', 99))


def build_nc():
    nc = bacc.Bacc("TRN2", target_bir_lowering=False, debug=False,
                   num_devices=N_CORES)
    xq = nc.declare_dram_parameter("xq", [D, TOK], F32, isOutput=False)
    xk = nc.declare_dram_parameter("xk", [D, TOK], F32, isOutput=False)
    xv = nc.declare_dram_parameter("xv", [D, TOK], F32, isOutput=False)
    wq = nc.declare_dram_parameter("wq", [D, D], BF16, isOutput=False)
    wk = nc.declare_dram_parameter("wk", [D, D], BF16, isOutput=False)
    wv = nc.declare_dram_parameter("wv", [D, D], BF16, isOutput=False)
    wo = nc.declare_dram_parameter("wo", [D, D], BF16, isOutput=False)
    m32 = nc.declare_dram_parameter("m32", [128, TOK // 128], F32, isOutput=False)
    ident = nc.declare_dram_parameter("ident", [128, 128], BF16, isOutput=False)
    out = nc.declare_dram_parameter("out", [D, TOK], F32, isOutput=True)

    from contextlib import ExitStack
    with TileContext(nc) as tc, ExitStack() as ctx:
        const = ctx.enter_context(tc.tile_pool(name="const", bufs=1))
        p_xb = ctx.enter_context(tc.tile_pool(name="xb", bufs=8))
        p_maj = ctx.enter_context(tc.tile_pool(name="maj", bufs=2))
        p_cc = ctx.enter_context(tc.tile_pool(name="cc", bufs=8))
        p_exp = ctx.enter_context(tc.tile_pool(name="expp", bufs=3))
        p_shs = ctx.enter_context(tc.tile_pool(name="shs", bufs=2))
        p_rcp = ctx.enter_context(tc.tile_pool(name="rcp", bufs=2))
        p_outT = ctx.enter_context(tc.tile_pool(name="outT", bufs=2))
        ps_proj = ctx.enter_context(tc.tile_pool(name="psp", bufs=2, space="PSUM"))
        ps_E = ctx.enter_context(tc.tile_pool(name="psE", bufs=3, space="PSUM"))
        ps_2 = ctx.enter_context(tc.tile_pool(name="ps2", bufs=2, space="PSUM"))
        p_stage = ctx.enter_context(tc.tile_pool(name="stage", bufs=2,
                                                 space="DRAM"))

        # ---- static tiles ----
        def load_w(name, dram):
            tiles = []
            for i in range(8):
                t = const.tile([128, D], BF16, tag=f"{name}{i}")
                nc.sync.dma_start(out=t[:], in_=dram[i * 128:(i + 1) * 128, :])
                tiles.append(t)
            return tiles

        wq_sb, wk_sb, wv_sb, wo_sb = (load_w(n, d) for n, d in
                                      (("wq", wq), ("wk", wk), ("wv", wv), ("wo", wo)))
        m32_sb = const.tile([128, TOK // 128], F32, tag="m32")
        nc.sync.dma_start(out=m32_sb[:], in_=m32[:])
        # Packed per-tc4 shuffle tiles (2x ping-pong, zeros static).
        # stqT [32=(b,h), (gf g2 i)]; bdkT [32=(b,h), (gf g2 y j)]
        # block-diagonal in (b,y); bdvT [128=(b,j), 64g*34(b',h | ones)].
        stqT_pp, bdkT_pp, bdvT_pp = [], [], []
        for i in range(2):
            t = const.tile([32, 64 * 64], BF16, tag=f"stqT{i}")
            stqT_pp.append(t)
            t = const.tile([32, 64 * 128], BF16, tag=f"bdkT{i}")
            nc.vector.memset(t[:], 0.0)
            bdkT_pp.append(t)
            t = const.tile([128, 64 * 34], BF16, tag=f"bdvT{i}")
            nc.vector.memset(t[:], 0.0)
            for b in range(2):
                # ones column at (b',h)-col 32+b for row-half b
                nc.vector.memset(
                    t[b * 64:(b + 1) * 64, :].rearrange(
                        "j (g c) -> j g c", c=34)[:, :, 32 + b:33 + b], 1.0)
            bdvT_pp.append(t)

        Copy = mybir.ActivationFunctionType.Copy
        Exp = mybir.ActivationFunctionType.Exp
        Mult = mybir.AluOpType.mult

        for mt in range(NMT):
            t0 = mt * MT
            # ---- load x megatile, cast to bf16 (SWDGE casts in flight) ----
            def load_x(dram, name):
                sbs = []
                for kc in range(8):
                    tb = p_xb.tile([128, MT], BF16, tag=f"x{name}")
                    nc.gpsimd.dma_start(out=tb[:],
                                        in_=dram[kc * 128:(kc + 1) * 128,
                                                 t0:t0 + MT])
                    sbs.append(tb)
                return sbs

            xq_sb = load_x(xq, "q")
            xk_sb = load_x(xk, "k")
            xv_sb = load_x(xv, "v")

            # ---- projections (T-major: out[t_chunk, o]) ----
            qmaj, kmaj, vmaj = [], [], []
            for tc4 in range(4):
                qm = p_maj.tile([128, D], BF16, tag="qmaj")
                km = p_maj.tile([128, D], BF16, tag="kmaj")
                vm = p_maj.tile([128, D], BF16, tag="vmaj")
                for dst, xsb, wsb, is_k in ((qm, xq_sb, wq_sb, False),
                                            (km, xk_sb, wk_sb, True),
                                            (vm, xv_sb, wv_sb, False)):
                    pss = [ps_proj.tile([128, 512], F32, tag="psp",
                                        name=f"psp{mt}_{tc4}_{id(dst)}_{i}")
                           for i in range(2)]
                    for kc in range(8):
                        for oc2 in range(2):
                            nc.tensor.matmul(
                                out=pss[oc2][:],
                                lhsT=xsb[kc][:, tc4 * 128:(tc4 + 1) * 128],
                                rhs=wsb[kc][:, oc2 * 512:(oc2 + 1) * 512],
                                start=(kc == 0), stop=(kc == 7))
                    for oc2 in range(2):
                        dslice = dst[:, oc2 * 512:(oc2 + 1) * 512]
                        if is_k:
                            mcol = mt * 4 + tc4
                            nc.vector.tensor_scalar(
                                out=dslice, in0=pss[oc2][:],
                                scalar1=m32_sb[:, mcol:mcol + 1], scalar2=None,
                                op0=Mult)
                        else:
                            nc.vector.tensor_copy(dslice, pss[oc2][:])
                qmaj.append(qm)
                kmaj.append(km)
                vmaj.append(vm)

            if STAGE <= 1:
                for oc in range(8):
                    nc.gpsimd.dma_start(out=out[oc * 128:(oc + 1) * 128,
                                                t0:t0 + MT],
                                        in_=qmaj[oc % 4][:, 0:512])
                continue

            # ---- attention ----
            concatT = [p_cc.tile([128, MT], F32, tag="cc", name=f"cc{mt}_{i}")
                       for i in range(8)]
            rcp64 = p_rcp.tile([64, MT], F32, tag="rcp64")
            sh = p_stage.tile([16, 64, MT], BF16, tag="sh")  # [h][i][t]
            for tc4 in range(4):
                pp = tc4 % 2
                stqT, bdkT, bdvT = stqT_pp[pp], bdkT_pp[pp], bdvT_pp[pp]
                # -- stage Q/K/V through DRAM: plain token-major dumps
                #    (1 contiguous DMA out); all shuffling on the read-back
                #    side as 3D-collapsible strided reads --
                sq = p_stage.tile([128, 1024], BF16, tag="sq")  # [t, (h i)]
                sk = p_stage.tile([128, 1024], BF16, tag="sk")  # [t, (h j)]
                sv = p_stage.tile([128, 1024], BF16, tag="sv")  # [t, (j h)]
                nc.sync.dma_start(out=sq[:], in_=qmaj[tc4][:])
                nc.sync.dma_start(out=sk[:], in_=kmaj[tc4][:])
                nc.sync.dma_start(out=sv[:], in_=vmaj[tc4][:])
                # token within tc4: t = gf*4 + g2*2 + b
                sqv = sq.rearrange("(gf g2 b) (h i) -> b h (gf g2) i",
                                   gf=32, g2=2, i=64)
                skv = sk.rearrange("(gf g2 b) (h j) -> b h (gf g2) j",
                                   gf=32, g2=2, j=64)
                svv = sv.rearrange("(g b) (j h) -> b j g h", b=2, h=16)
                for b in range(2):
                    nc.sync.dma_start(
                        out=stqT[b * 16:(b + 1) * 16, :].rearrange(
                            "h (gf g2 i) -> h (gf g2) i", g2=2, i=64),
                        in_=sqv[b])
                    nc.sync.dma_start(
                        out=bdkT[b * 16:(b + 1) * 16, :].rearrange(
                            "h (gf g2 y j) -> h (gf g2) y j",
                            g2=2, y=2, j=64)[:, :, b, :],
                        in_=skv[b])
                    nc.sync.dma_start(
                        out=bdvT[b * 64:(b + 1) * 64, :].rearrange(
                            "j (g c) -> j g c", c=34)[:, :, b * 16:(b + 1) * 16],
                        in_=svv[b])
                if STAGE <= 2:
                    continue
                # per-tc4 head accumulator [i, h, t_local] bf16
                sh_sb = p_shs.tile([64, 16, 128], BF16, tag="shs")
                for batch in range(8):  # 16 tokens
                    bt = tc4 * 8 + batch
                    psE = ps_E.tile([128, 512], F32, tag="psE")
                    for g8 in range(8):
                        g = batch * 8 + g8      # group in tc4 (2 tokens)
                        nc.tensor.matmul(
                            out=psE[:, g8 * 64:(g8 + 1) * 64],
                            lhsT=bdkT[:, g * 128:(g + 1) * 128],
                            rhs=stqT[:, g * 64:(g + 1) * 64],
                            start=True, stop=True)
                    expE = p_exp.tile([128, 512], BF16, tag="expE")
                    nc.scalar.activation(out=expE[:], in_=psE[:], func=Exp)
                    ps2 = ps_2.tile([64, 272], F32, tag="ps2")
                    for g8 in range(8):
                        g = batch * 8 + g8
                        nc.tensor.matmul(
                            out=ps2[:, g8 * 34:(g8 + 1) * 34],
                            lhsT=expE[:, g8 * 64:(g8 + 1) * 64],
                            rhs=bdvT[:, g * 34:(g + 1) * 34],
                            start=True, stop=True)
                    ps2v = ps2[:].rearrange("p (g c) -> p g c", c=34)
                    nc.vector.reciprocal(
                        rcp64[:, bt * 16:(bt + 1) * 16].rearrange(
                            "p (g b) -> p g b", b=2),
                        ps2v[:, :, 32:34])
                    # head rows into per-tc4 accumulator: free = (h, t16)
                    nc.vector.tensor_copy(
                        sh_sb[:, :, batch * 16:(batch + 1) * 16].rearrange(
                            "p h (g b) -> p g b h", g=8, b=2),
                        ps2v[:, :, 0:32].rearrange("p g (b h) -> p g b h", h=16))
                # one staged write per tc4: [h][i][128t]
                nc.gpsimd.dma_start(
                    out=sh[:, :, tc4 * 128:(tc4 + 1) * 128].rearrange(
                        "h i t -> i h t"),
                    in_=sh_sb[:])
            if STAGE <= 2:
                for oc in range(8):
                    nc.gpsimd.dma_start(out=out[oc * 128:(oc + 1) * 128,
                                                t0:t0 + MT],
                                        in_=bdkT_pp[oc % 2][:, 0:1024])
                continue
            for kc in range(8):
                nc.gpsimd.dma_start(out=concatT[kc][:],
                                    in_=sh[2 * kc:2 * kc + 2])
            if STAGE <= 3:
                for oc in range(8):
                    nc.sync.dma_start(out=out[oc * 128:(oc + 1) * 128,
                                              t0:t0 + MT],
                                      in_=concatT[oc][:])
                continue

            # ---- normalize + output projection ----
            rcp128 = p_rcp.tile([128, MT], F32, tag="rcp128")
            nc.vector.tensor_copy(rcp128[0:64, :], rcp64[:])
            nc.sync.dma_start(out=rcp128[64:128, :], in_=rcp64[:])
            ccb = []
            for kc in range(8):
                cb = p_cc.tile([128, MT], BF16, tag="ccb")
                nc.vector.tensor_tensor(out=cb[:], in0=concatT[kc][:],
                                        in1=rcp128[:], op=Mult)
                ccb.append(cb)
            for oc in range(8):
                ps = ps_proj.tile([128, 512], F32, tag="psp")
                for kc in range(8):
                    nc.tensor.matmul(out=ps[:],
                                     lhsT=wo_sb[kc][:, oc * 128:(oc + 1) * 128],
                                     rhs=ccb[kc][:],
                                     start=(kc == 0), stop=(kc == 7))
                ot = p_outT.tile([128, MT], F32, tag="outT")
                nc.vector.tensor_copy(ot[:], ps[:])
                nc.sync.dma_start(out=out[oc * 128:(oc + 1) * 128, t0:t0 + MT],
                                  in_=ot[:])
    nc.compile()
    return nc


_NC_CACHE = None


def _get_nc():
    global _NC_CACHE
    if _NC_CACHE is None:
        _NC_CACHE = build_nc()
    return _NC_CACHE


def _host_prep(queries, keys, values, mask, Wq, Wk, Wv, Wo):
    """Build the 8 per-core input maps."""
    fq = np.ascontiguousarray(queries.reshape(N * T, D).T)  # [D, 16384]
    fk = np.ascontiguousarray(keys.reshape(N * T, D).T)
    fv = np.ascontiguousarray(values.reshape(N * T, D).T)
    fm = mask.reshape(N * T).astype(np.float32) / 32.0

    import ml_dtypes
    bf = lambda a: np.ascontiguousarray(a).astype(ml_dtypes.bfloat16)
    wq_h = bf(Wq.T)
    wk_h = bf(Wk.T)
    ov = np.arange(D)
    perm_v = (ov % 16) * 64 + (ov // 16)  # device col j*16+h <- orig h*64+j
    wv_h = bf(Wv.T[:, perm_v])
    kpp = np.arange(D)
    perm = (kpp % 64) * 16 + (kpp // 64)  # k''=h*64+i -> source row i*16+h
    wo_h = bf(Wo.T[perm])
    ident = np.eye(128, dtype=np.float32).astype(ml_dtypes.bfloat16)

    in_maps = []
    for c in range(N_CORES):
        s = slice(c * TOK, (c + 1) * TOK)
        in_maps.append({
            "xq": np.ascontiguousarray(fq[:, s]),
            "xk": np.ascontiguousarray(fk[:, s]),
            "xv": np.ascontiguousarray(fv[:, s]),
            "wq": wq_h, "wk": wk_h, "wv": wv_h, "wo": wo_h,
            "m32": np.ascontiguousarray(fm[s].reshape(TOK // 128, 128).T),
            "ident": ident,
        })
    return in_maps


def kernel(queries, keys, values, mask, Wq, Wk, Wv, Wo, _trace=False, _tmpdir=None):
    queries = np.asarray(queries, dtype=np.float32)
    keys = np.asarray(keys, dtype=np.float32)
    values = np.asarray(values, dtype=np.float32)
    mask = np.asarray(mask)
    in_maps = _host_prep(queries, keys, values, mask,
                         np.asarray(Wq, np.float32), np.asarray(Wk, np.float32),
                         np.asarray(Wv, np.float32), np.asarray(Wo, np.float32))
    nc = _get_nc()
    res = run_bass_kernel_spmd(nc, in_maps, core_ids=list(range(N_CORES)),
                               trace=_trace, tmpdir=_tmpdir)
    outs = []
    for c in range(N_CORES):
        outs.append(np.asarray(res.results[c]["out"]).T)  # [TOK, D]
    full = np.concatenate(outs, axis=0).reshape(N, T, D)
    kernel.last_exec_time_ns = res.exec_time_ns
    return full
